# revision 20
# baseline (speedup 1.0000x reference)
import sys

if "/opt/trn_rl_repo" not in sys.path:
    sys.path.insert(0, "/opt/trn_rl_repo")

import numpy as np

B, HD, H, W, K = 2, 4, 128, 128, 49
KS = 7
NSP = 9
S = 64
N_CORES = 8
WQ = W // 4            # 32 columns per core
CH = 4                 # columns per compute chunk
NCH = WQ // CH         # chunks
HD_K = HD * K          # 196
SK = NSP * K           # 441
U_SZ = HD * NSP * K    # 1764

_cached = {}


def _build():
    import concourse.bass as bass
    import concourse.tile as tile
    from concourse import bacc, mybir

    f32 = mybir.dt.float32
    bf16 = mybir.dt.bfloat16
    mult = mybir.AluOpType.mult
    add = mybir.AluOpType.add

    nc = bacc.Bacc("TRN2", target_bir_lowering=False, debug=False, num_devices=N_CORES)
    # e9[h, c, hd, s, k] = exp(logits) broadcast over s (bf16, host-expanded)
    e9_s = nc.dram_tensor("e9", [128, WQ * U_SZ], bf16, kind="ExternalInput")
    # pg[h, c, s, k] gathered patches (bf16, dense 441/col)
    pg_s = nc.dram_tensor("pg", [128, WQ * SK], bf16, kind="ExternalInput")
    # pi[h, c, s] (bf16)
    pi_s = nc.dram_tensor("pi", [128, WQ * NSP], bf16, kind="ExternalInput")
    # out[h, c, hd, k] bf16 (host converts to f32)
    out_s = nc.dram_tensor("out", [128, WQ * HD_K], bf16, kind="ExternalOutput")

    def ap(t, off, dims):
        return bass.AP(t, off, [list(d) for d in dims])

    def sap(tap, extra_off, dims):
        return bass.AP(tap.tensor, tap.offset + extra_off, [list(tap.ap[0]), *[list(d) for d in dims]])

    with tile.TileContext(nc) as tc:
        with (
            tc.tile_pool(name="e9p", bufs=3) as e9p,
            tc.tile_pool(name="pgp", bufs=3) as pgp,
            tc.tile_pool(name="pip", bufs=1) as pip,
            tc.tile_pool(name="up", bufs=2) as up,
            tc.tile_pool(name="sp", bufs=4) as sp,
            tc.tile_pool(name="dp", bufs=2) as dp,
            tc.tile_pool(name="acp", bufs=2) as acp,
            tc.tile_pool(name="tp", bufs=2) as tp,
            tc.tile_pool(name="op", bufs=3) as op,
        ):
            pi_t = pip.tile([128, WQ * NSP], bf16, tag="pi")
            nc.sync.dma_start(pi_t[:], ap(pi_s, 0, [(WQ * NSP, 128), (1, WQ * NSP)]))

            def emit_tail(st):
                acc4, wl0 = st
                # o[hd,k] = sum_s acc via packed bf16 add-tree
                t1 = tp.tile([128, CH * HD * 4 * K], bf16, tag="t1")
                nc.vector.tensor_tensor(
                    out=sap(t1[:], 0, [(784, CH), (196, HD), (1, 4 * K)]),
                    in0=sap(acc4[:], 0, [(U_SZ, CH), (441, HD), (1, 4 * K)]),
                    in1=sap(acc4[:], 4 * K, [(U_SZ, CH), (441, HD), (1, 4 * K)]),
                    op=add,
                )
                t2 = tp.tile([128, CH * HD * 2 * K], bf16, tag="t2")
                nc.vector.tensor_tensor(
                    out=sap(t2[:], 0, [(392, CH), (98, HD), (1, 2 * K)]),
                    in0=sap(t1[:], 0, [(784, CH), (196, HD), (1, 2 * K)]),
                    in1=sap(t1[:], 2 * K, [(784, CH), (196, HD), (1, 2 * K)]),
                    op=add,
                )
                t3 = tp.tile([128, CH * HD_K], bf16, tag="t3")
                nc.vector.tensor_tensor(
                    out=sap(t3[:], 0, [(196, CH), (49, HD), (1, K)]),
                    in0=sap(t2[:], 0, [(392, CH), (98, HD), (1, K)]),
                    in1=sap(t2[:], K, [(392, CH), (98, HD), (1, K)]),
                    op=add,
                )
                o4 = op.tile([128, CH * HD_K], bf16, tag="o")
                nc.vector.tensor_tensor(
                    out=sap(o4[:], 0, [(196, CH), (49, HD), (1, K)]),
                    in0=sap(t3[:], 0, [(196, CH), (49, HD), (1, K)]),
                    in1=sap(acc4[:], 8 * K, [(U_SZ, CH), (441, HD), (1, K)]),
                    op=add,
                )
                nc.sync.dma_start(
                    ap(out_s, wl0 * HD_K, [(WQ * HD_K, 128), (HD_K, CH), (1, HD_K)]),
                    o4[:],
                )

            pend = None
            for ch in range(NCH):
                wl0 = ch * CH

                pg_t = pgp.tile([128, CH * SK], bf16, tag="pg")
                nc.sync.dma_start(
                    pg_t[:],
                    ap(pg_s, wl0 * SK, [(WQ * SK, 128), (1, CH * SK)]),
                )
                e9_t = e9p.tile([128, CH * U_SZ], bf16, tag="e9")
                HB = CH * U_SZ // 2
                nc.sync.dma_start(
                    sap(e9_t[:], 0, [(1, HB)]),
                    ap(e9_s, wl0 * U_SZ, [(WQ * U_SZ, 128), (1, HB)]),
                )
                nc.sync.dma_start(
                    sap(e9_t[:], HB, [(1, HB)]),
                    ap(e9_s, wl0 * U_SZ + HB, [(WQ * U_SZ, 128), (1, HB)]),
                )

                # u[c,hd,s,k] = e9[c,hd,s,k] * p[c,s,k]  (one merged bf16 2x TT)
                u4 = up.tile([128, CH * U_SZ], bf16, tag="u")
                nc.vector.tensor_tensor(
                    out=sap(u4[:], 0, [(U_SZ, CH), (441, HD), (1, SK)]),
                    in0=sap(e9_t[:], 0, [(U_SZ, CH), (441, HD), (1, SK)]),
                    in1=sap(pg_t[:], 0, [(SK, CH), (0, HD), (1, SK)]),
                    op=mult,
                )

                # d[c,hd,s] = sum_k u: two packed bf16 halvings, reduce,
                # then add the k=48 leftover lane (fp32)
                a1 = dp.tile([128, CH * 36 * 24], bf16, tag="a1")
                nc.vector.tensor_tensor(
                    out=sap(a1[:], 0, [(864, CH), (24, 36), (1, 24)]),
                    in0=sap(u4[:], 0, [(U_SZ, CH), (K, 36), (1, 24)]),
                    in1=sap(u4[:], 24, [(U_SZ, CH), (K, 36), (1, 24)]),
                    op=add,
                )
                a2 = dp.tile([128, CH * 36 * 12], bf16, tag="a2")
                nc.vector.tensor_tensor(
                    out=sap(a2[:], 0, [(432, CH), (12, 36), (1, 12)]),
                    in0=sap(a1[:], 0, [(864, CH), (24, 36), (1, 12)]),
                    in1=sap(a1[:], 12, [(864, CH), (24, 36), (1, 12)]),
                    op=add,
                )
                d0 = sp.tile([128, CH * HD * NSP], f32, tag="d0")
                nc.vector.reduce_sum(
                    out=sap(d0[:], 0, [(36, CH), (1, 36)]),
                    in_=sap(a2[:], 0, [(432, CH), (12, 36), (1, 12)]),
                    axis=mybir.AxisListType.X,
                )
                d4 = sp.tile([128, CH * HD * NSP], f32, tag="d")
                nc.vector.tensor_tensor(
                    out=sap(d4[:], 0, [(36, CH), (1, 36)]),
                    in0=sap(d0[:], 0, [(36, CH), (1, 36)]),
                    in1=sap(u4[:], 48, [(U_SZ, CH), (K, 36)]),
                    op=add,
                )

                # r = 1/d fast approx; v[c,hd,s] = r * pi[c,s]  (bf16)
                r4 = sp.tile([128, CH * HD * NSP], f32, tag="r")
                nc.vector.reciprocal_approx_fast(r4[:], d4[:])
                v4 = sp.tile([128, CH * HD * NSP], bf16, tag="v")
                nc.vector.tensor_tensor(
                    out=sap(v4[:], 0, [(36, CH), (NSP, HD), (1, NSP)]),
                    in0=sap(r4[:], 0, [(36, CH), (NSP, HD), (1, NSP)]),
                    in1=sap(pi_t[:], wl0 * NSP, [(NSP, CH), (0, HD), (1, NSP)]),
                    op=mult,
                )

                # vk[c,hd,s,k] = v broadcast along k, on the Act engine
                wk4 = acp.tile([128, CH * U_SZ], bf16, tag="wk")
                nc.scalar.activation(
                    sap(wk4[:], 0, [(K, CH * 36), (1, K)]),
                    sap(v4[:], 0, [(1, CH * 36), (0, K)]),
                    mybir.ActivationFunctionType.Copy,
                )

                # o-tree of the previous chunk fills the wk wait window
                if pend is not None:
                    emit_tail(pend)

                # acc[c,hd,s,k] = u * vk  (bf16 2x on DVE)
                acc4 = acp.tile([128, CH * U_SZ], bf16, tag="acc")
                nc.vector.tensor_tensor(
                    out=sap(acc4[:], 0, [(1, CH * U_SZ)]),
                    in0=sap(u4[:], 0, [(1, CH * U_SZ)]),
                    in1=sap(wk4[:], 0, [(1, CH * U_SZ)]),
                    op=mult,
                )

                pend = (acc4, wl0)
            emit_tail(pend)
    nc.compile()
    return nc


def _host_prep(attn, sims, sinds):
    from concourse import mybir

    bf_np = mybir.dt.np(mybir.dt.bfloat16)
    hj = (np.clip(np.arange(H) - KS // 2, 0, H - KS)[:, None] + np.arange(KS)[None, :])
    wj = (np.clip(np.arange(W) - KS // 2, 0, W - KS)[:, None] + np.arange(KS)[None, :])
    harange = np.arange(H)
    in_maps = []
    for b in range(B):
        sims_b = sims[b]                                  # (S,H,W)
        for q in range(4):
            cols = np.arange(WQ * q, WQ * (q + 1))
            e = np.exp(np.ascontiguousarray(
                attn[b][:, :, cols, :].transpose(1, 2, 0, 3)
            ))                                            # (H,WQ,HD,K)
            e9 = np.broadcast_to(
                e[:, :, :, None, :], (H, WQ, HD, NSP, K)
            ).reshape(128, WQ * U_SZ).astype(bf_np)

            g = sinds[b][:, cols, :]                      # (H,WQ,9)
            patch = sims_b[
                g[:, :, :, None, None],
                hj[:, None, None, :, None],
                wj[cols][None, :, None, None, :],
            ]                                             # (H,WQ,9,7,7)
            pi = sims_b[g, harange[:, None, None], cols[None, :, None]]
            in_maps.append({
                "e9": e9,
                "pg": patch.reshape(128, WQ * SK).astype(bf_np),
                "pi": pi.reshape(128, WQ * NSP).astype(bf_np),
            })
    return in_maps


def kernel(attn, sims, sinds):
    from concourse.bass_utils import run_bass_kernel_spmd

    attn = np.asarray(attn, dtype=np.float32)
    sims = np.asarray(sims, dtype=np.float32)
    sinds = np.asarray(sinds)

    if "nc" not in _cached:
        _cached["nc"] = _build()
    nc = _cached["nc"]

    in_maps = _host_prep(attn, sims, sinds)
    res = run_bass_kernel_spmd(nc, in_maps, list(range(N_CORES)))

    out = np.empty((B, HD, H, W, K), dtype=np.float32)
    for cid in range(N_CORES):
        b, q = divmod(cid, 4)
        o = res.results[cid]["out"].astype(np.float32).reshape(H, WQ, HD, K)
        out[b][:, :, WQ * q:WQ * (q + 1), :] = o.transpose(2, 0, 1, 3)
    return out


# revision 21
# speedup vs baseline: 1.0207x; 1.0207x over previous
import sys

if "/opt/trn_rl_repo" not in sys.path:
    sys.path.insert(0, "/opt/trn_rl_repo")

import numpy as np

B, HD, H, W, K = 2, 4, 128, 128, 49
KS = 7
NSP = 9
S = 64
N_CORES = 8
WQ = W // 4            # 32 columns per core
PGE = 50               # per-(pixel,s) gathered element: 49 patch + pi
NQ = 4                 # input DMA quarters
CPQ = WQ // NQ         # columns per quarter
CH = 4                 # columns per compute chunk
NCH = WQ // CH         # chunks
HD_K = HD * K          # 196
U_SZ = HD * NSP * K    # 1764

_cached = {}


def _build():
    import concourse.bass as bass
    import concourse.tile as tile
    from concourse import bacc, mybir

    f32 = mybir.dt.float32
    bf16 = mybir.dt.bfloat16
    mult = mybir.AluOpType.mult
    add = mybir.AluOpType.add

    nc = bacc.Bacc("TRN2", target_bir_lowering=False, debug=False, num_devices=N_CORES)
    # e2[h, wl, hd, k] = exp(logits) (bf16, host precomputed)
    attn_s = nc.dram_tensor("attn", [128, WQ * HD_K], bf16, kind="ExternalInput")
    # pg[h, wl, s, 0:49]=patch, [...,49]=pi (bf16, host pre-gathered)
    pg_s = nc.dram_tensor("pg", [128, WQ * NSP * PGE], bf16, kind="ExternalInput")
    # out[h, wl, hd, k] f32
    out_s = nc.dram_tensor("out", [128, WQ * HD_K], bf16, kind="ExternalOutput")

    def ap(t, off, dims):
        return bass.AP(t, off, [list(d) for d in dims])

    def sap(tap, extra_off, dims):
        return bass.AP(tap.tensor, tap.offset + extra_off, [list(tap.ap[0]), *[list(d) for d in dims]])

    with tile.TileContext(nc) as tc:
        with (
            tc.tile_pool(name="inq", bufs=NQ) as inq,
            tc.tile_pool(name="ep", bufs=2) as epool,
            tc.tile_pool(name="up", bufs=2) as up,
            tc.tile_pool(name="sp", bufs=4) as sp,
            tc.tile_pool(name="dp", bufs=3) as dp,
            tc.tile_pool(name="acp", bufs=2) as acp,
            tc.tile_pool(name="tp", bufs=3) as tp,
            tc.tile_pool(name="op", bufs=3) as op,
        ):
            at_q = []
            pg_q = []
            for qi in range(NQ):
                at_t = inq.tile([128, CPQ * HD_K], bf16, tag="atq")
                nc.sync.dma_start(
                    at_t[:],
                    ap(attn_s, qi * CPQ * HD_K, [(WQ * HD_K, 128), (1, CPQ * HD_K)]),
                )
                pg_t = inq.tile([128, CPQ * NSP * PGE], bf16, tag="pgq")
                nc.sync.dma_start(
                    pg_t[:],
                    ap(pg_s, qi * CPQ * NSP * PGE, [(WQ * NSP * PGE, 128), (1, CPQ * NSP * PGE)]),
                )
                at_q.append(at_t)
                pg_q.append(pg_t)

            def emit_tail(st):
                acc4, wl0 = st
                # o[hd,k] = sum_s acc via packed bf16 add-tree
                t1 = tp.tile([128, CH * HD * 4 * K], bf16, tag="t1")
                nc.vector.tensor_tensor(
                    out=sap(t1[:], 0, [(784, CH), (196, HD), (1, 4 * K)]),
                    in0=sap(acc4[:], 0, [(U_SZ, CH), (441, HD), (1, 4 * K)]),
                    in1=sap(acc4[:], 4 * K, [(U_SZ, CH), (441, HD), (1, 4 * K)]),
                    op=add,
                )
                t2 = tp.tile([128, CH * HD * 2 * K], bf16, tag="t2")
                nc.vector.tensor_tensor(
                    out=sap(t2[:], 0, [(392, CH), (98, HD), (1, 2 * K)]),
                    in0=sap(t1[:], 0, [(784, CH), (196, HD), (1, 2 * K)]),
                    in1=sap(t1[:], 2 * K, [(784, CH), (196, HD), (1, 2 * K)]),
                    op=add,
                )
                t3 = tp.tile([128, CH * HD_K], bf16, tag="t3")
                nc.vector.tensor_tensor(
                    out=sap(t3[:], 0, [(196, CH), (49, HD), (1, K)]),
                    in0=sap(t2[:], 0, [(392, CH), (98, HD), (1, K)]),
                    in1=sap(t2[:], K, [(392, CH), (98, HD), (1, K)]),
                    op=add,
                )
                o4 = op.tile([128, CH * HD_K], bf16, tag="o")
                nc.vector.tensor_tensor(
                    out=sap(o4[:], 0, [(196, CH), (49, HD), (1, K)]),
                    in0=sap(t3[:], 0, [(196, CH), (49, HD), (1, K)]),
                    in1=sap(acc4[:], 8 * K, [(U_SZ, CH), (441, HD), (1, K)]),
                    op=add,
                )
                nc.sync.dma_start(
                    ap(out_s, wl0 * HD_K, [(WQ * HD_K, 128), (HD_K, CH), (1, HD_K)]),
                    o4[:],
                )

            pend = None
            for ch in range(NCH):
                wl0 = ch * CH
                qi, jo = divmod(wl0, CPQ)
                at_off = jo * HD_K
                pg_off = jo * NSP * PGE

                # u[c,hd,s,k] = e[c,hd,k] * p[c,s,k]  (bf16 2x; per-column —
                # the broadcast dims don't fit the 3-free-dim ISA limit)
                u4 = up.tile([128, CH * U_SZ], bf16, tag="u")
                for j in range(CH):
                    nc.vector.tensor_tensor(
                        out=sap(u4[:], j * U_SZ, [(441, HD), (K, NSP), (1, K)]),
                        in0=sap(at_q[qi][:], at_off + j * HD_K, [(K, HD), (0, NSP), (1, K)]),
                        in1=sap(pg_q[qi][:], pg_off + j * NSP * PGE, [(0, HD), (PGE, NSP), (1, K)]),
                        op=mult,
                    )

                # d[c,hd,s] = sum_k u: one packed bf16 halving, then reduce,
                # then add the k=48 leftover lane (fp32)
                a1 = dp.tile([128, CH * 36 * 24], bf16, tag="a1")
                nc.vector.tensor_tensor(
                    out=sap(a1[:], 0, [(864, CH), (24, 36), (1, 24)]),
                    in0=sap(u4[:], 0, [(U_SZ, CH), (K, 36), (1, 24)]),
                    in1=sap(u4[:], 24, [(U_SZ, CH), (K, 36), (1, 24)]),
                    op=add,
                )
                a2 = dp.tile([128, CH * 36 * 12], bf16, tag="a2")
                nc.vector.tensor_tensor(
                    out=sap(a2[:], 0, [(432, CH), (12, 36), (1, 12)]),
                    in0=sap(a1[:], 0, [(864, CH), (24, 36), (1, 12)]),
                    in1=sap(a1[:], 12, [(864, CH), (24, 36), (1, 12)]),
                    op=add,
                )
                d0 = sp.tile([128, CH * HD * NSP], f32, tag="d0")
                nc.vector.reduce_sum(
                    out=sap(d0[:], 0, [(36, CH), (1, 36)]),
                    in_=sap(a2[:], 0, [(432, CH), (12, 36), (1, 12)]),
                    axis=mybir.AxisListType.X,
                )
                d4 = sp.tile([128, CH * HD * NSP], f32, tag="d")
                nc.vector.tensor_tensor(
                    out=sap(d4[:], 0, [(36, CH), (1, 36)]),
                    in0=sap(d0[:], 0, [(36, CH), (1, 36)]),
                    in1=sap(u4[:], 48, [(U_SZ, CH), (K, 36)]),
                    op=add,
                )

                # r = 1/d fast approx; v[c,hd,s] = r * pi[c,s]  (bf16)
                r4 = sp.tile([128, CH * HD * NSP], f32, tag="r")
                nc.vector.reciprocal_approx_fast(r4[:], d4[:])
                v4 = sp.tile([128, CH * HD * NSP], bf16, tag="v")
                nc.vector.tensor_tensor(
                    out=sap(v4[:], 0, [(36, CH), (NSP, HD), (1, NSP)]),
                    in0=sap(r4[:], 0, [(36, CH), (NSP, HD), (1, NSP)]),
                    in1=sap(pg_q[qi][:], pg_off + K, [(NSP * PGE, CH), (0, HD), (PGE, NSP)]),
                    op=mult,
                )

                # vk[c,hd,s,k] = v broadcast along k, on the Act engine
                wk4 = acp.tile([128, CH * U_SZ], bf16, tag="wk")
                nc.scalar.activation(
                    sap(wk4[:], 0, [(K, CH * 36), (1, K)]),
                    sap(v4[:], 0, [(1, CH * 36), (0, K)]),
                    mybir.ActivationFunctionType.Copy,
                )
                # acc[c,hd,s,k] = u * vk  (bf16 2x on DVE)
                acc4 = acp.tile([128, CH * U_SZ], bf16, tag="acc")
                nc.vector.tensor_tensor(
                    out=sap(acc4[:], 0, [(1, CH * U_SZ)]),
                    in0=sap(u4[:], 0, [(1, CH * U_SZ)]),
                    in1=sap(wk4[:], 0, [(1, CH * U_SZ)]),
                    op=mult,
                )

                if pend is not None:
                    emit_tail(pend)
                pend = (acc4, wl0)
            emit_tail(pend)
    nc.compile()
    return nc


def _host_prep(attn, sims, sinds):
    from concourse import mybir

    bf_np = mybir.dt.np(mybir.dt.bfloat16)
    hj = (np.clip(np.arange(H) - KS // 2, 0, H - KS)[:, None] + np.arange(KS)[None, :])
    wj = (np.clip(np.arange(W) - KS // 2, 0, W - KS)[:, None] + np.arange(KS)[None, :])
    harange = np.arange(H)
    in_maps = []
    for b in range(B):
        sims_b = sims[b]                                  # (S,H,W)
        for q in range(4):
            cols = np.arange(WQ * q, WQ * (q + 1))
            attn2 = np.exp(np.ascontiguousarray(
                attn[b][:, :, cols, :].transpose(1, 2, 0, 3)
            )).reshape(128, WQ * HD_K).astype(bf_np)

            g = sinds[b][:, cols, :]                      # (H,WQ,9)
            patch = sims_b[
                g[:, :, :, None, None],
                hj[:, None, None, :, None],
                wj[cols][None, :, None, None, :],
            ]                                             # (H,WQ,9,7,7)
            pi = sims_b[g, harange[:, None, None], cols[None, :, None]]
            pg = np.empty((H, WQ, NSP, PGE), dtype=np.float32)
            pg[..., :K] = patch.reshape(H, WQ, NSP, K)
            pg[..., K] = pi
            in_maps.append({
                "attn": attn2,
                "pg": pg.reshape(128, WQ * NSP * PGE).astype(bf_np),
            })
    return in_maps


def kernel(attn, sims, sinds):
    from concourse.bass_utils import run_bass_kernel_spmd

    attn = np.asarray(attn, dtype=np.float32)
    sims = np.asarray(sims, dtype=np.float32)
    sinds = np.asarray(sinds)

    if "nc" not in _cached:
        _cached["nc"] = _build()
    nc = _cached["nc"]

    in_maps = _host_prep(attn, sims, sinds)
    res = run_bass_kernel_spmd(nc, in_maps, list(range(N_CORES)))

    out = np.empty((B, HD, H, W, K), dtype=np.float32)
    for cid in range(N_CORES):
        b, q = divmod(cid, 4)
        o = res.results[cid]["out"].astype(np.float32).reshape(H, WQ, HD, K)
        out[b][:, :, WQ * q:WQ * (q + 1), :] = o.transpose(2, 0, 1, 3)
    return out


# revision 22
# speedup vs baseline: 1.0271x; 1.0062x over previous
import sys

if "/opt/trn_rl_repo" not in sys.path:
    sys.path.insert(0, "/opt/trn_rl_repo")

import numpy as np

B, HD, H, W, K = 2, 4, 128, 128, 49
KS = 7
NSP = 9
S = 64
N_CORES = 8
WQ = W // 4            # 32 columns per core
PGE = 50               # per-(pixel,s) gathered element: 49 patch + pi
NQ = 8                 # input DMA groups (one per chunk)
CPQ = WQ // NQ         # columns per quarter
CH = 4                 # columns per compute chunk
NCH = WQ // CH         # chunks
HD_K = HD * K          # 196
U_SZ = HD * NSP * K    # 1764

_cached = {}


def _build():
    import concourse.bass as bass
    import concourse.tile as tile
    from concourse import bacc, mybir

    f32 = mybir.dt.float32
    bf16 = mybir.dt.bfloat16
    mult = mybir.AluOpType.mult
    add = mybir.AluOpType.add

    nc = bacc.Bacc("TRN2", target_bir_lowering=False, debug=False, num_devices=N_CORES)
    # e2[h, wl, hd, k] = exp(logits) (bf16, host precomputed)
    attn_s = nc.dram_tensor("attn", [128, WQ * HD_K], bf16, kind="ExternalInput")
    # pg[h, wl, s, 0:49]=patch, [...,49]=pi (bf16, host pre-gathered)
    pg_s = nc.dram_tensor("pg", [128, WQ * NSP * PGE], bf16, kind="ExternalInput")
    # out[h, wl, hd, k] f32
    out_s = nc.dram_tensor("out", [128, WQ * HD_K], bf16, kind="ExternalOutput")

    def ap(t, off, dims):
        return bass.AP(t, off, [list(d) for d in dims])

    def sap(tap, extra_off, dims):
        return bass.AP(tap.tensor, tap.offset + extra_off, [list(tap.ap[0]), *[list(d) for d in dims]])

    with tile.TileContext(nc) as tc:
        with (
            tc.tile_pool(name="inq", bufs=NQ) as inq,
            tc.tile_pool(name="ep", bufs=2) as epool,
            tc.tile_pool(name="up", bufs=2) as up,
            tc.tile_pool(name="sp", bufs=4) as sp,
            tc.tile_pool(name="dp", bufs=3) as dp,
            tc.tile_pool(name="acp", bufs=2) as acp,
            tc.tile_pool(name="tp", bufs=3) as tp,
            tc.tile_pool(name="op", bufs=3) as op,
        ):
            at_q = []
            pg_q = []
            for qi in range(NQ):
                at_t = inq.tile([128, CPQ * HD_K], bf16, tag="atq")
                nc.sync.dma_start(
                    at_t[:],
                    ap(attn_s, qi * CPQ * HD_K, [(WQ * HD_K, 128), (1, CPQ * HD_K)]),
                )
                pg_t = inq.tile([128, CPQ * NSP * PGE], bf16, tag="pgq")
                nc.sync.dma_start(
                    pg_t[:],
                    ap(pg_s, qi * CPQ * NSP * PGE, [(WQ * NSP * PGE, 128), (1, CPQ * NSP * PGE)]),
                )
                at_q.append(at_t)
                pg_q.append(pg_t)

            def emit_tail(st):
                acc4, wl0 = st
                # o[hd,k] = sum_s acc via packed bf16 add-tree
                t1 = tp.tile([128, CH * HD * 4 * K], bf16, tag="t1")
                nc.vector.tensor_tensor(
                    out=sap(t1[:], 0, [(784, CH), (196, HD), (1, 4 * K)]),
                    in0=sap(acc4[:], 0, [(U_SZ, CH), (441, HD), (1, 4 * K)]),
                    in1=sap(acc4[:], 4 * K, [(U_SZ, CH), (441, HD), (1, 4 * K)]),
                    op=add,
                )
                t2 = tp.tile([128, CH * HD * 2 * K], bf16, tag="t2")
                nc.vector.tensor_tensor(
                    out=sap(t2[:], 0, [(392, CH), (98, HD), (1, 2 * K)]),
                    in0=sap(t1[:], 0, [(784, CH), (196, HD), (1, 2 * K)]),
                    in1=sap(t1[:], 2 * K, [(784, CH), (196, HD), (1, 2 * K)]),
                    op=add,
                )
                t3 = tp.tile([128, CH * HD_K], bf16, tag="t3")
                nc.vector.tensor_tensor(
                    out=sap(t3[:], 0, [(196, CH), (49, HD), (1, K)]),
                    in0=sap(t2[:], 0, [(392, CH), (98, HD), (1, K)]),
                    in1=sap(t2[:], K, [(392, CH), (98, HD), (1, K)]),
                    op=add,
                )
                o4 = op.tile([128, CH * HD_K], bf16, tag="o")
                nc.vector.tensor_tensor(
                    out=sap(o4[:], 0, [(196, CH), (49, HD), (1, K)]),
                    in0=sap(t3[:], 0, [(196, CH), (49, HD), (1, K)]),
                    in1=sap(acc4[:], 8 * K, [(U_SZ, CH), (441, HD), (1, K)]),
                    op=add,
                )
                nc.sync.dma_start(
                    ap(out_s, wl0 * HD_K, [(WQ * HD_K, 128), (HD_K, CH), (1, HD_K)]),
                    o4[:],
                )

            pend = None
            for ch in range(NCH):
                wl0 = ch * CH
                qi, jo = divmod(wl0, CPQ)
                at_off = jo * HD_K
                pg_off = jo * NSP * PGE

                # u[c,hd,s,k] = e[c,hd,k] * p[c,s,k]  (bf16 2x; per-column —
                # the broadcast dims don't fit the 3-free-dim ISA limit)
                u4 = up.tile([128, CH * U_SZ], bf16, tag="u")
                for j in range(CH):
                    nc.vector.tensor_tensor(
                        out=sap(u4[:], j * U_SZ, [(441, HD), (K, NSP), (1, K)]),
                        in0=sap(at_q[qi][:], at_off + j * HD_K, [(K, HD), (0, NSP), (1, K)]),
                        in1=sap(pg_q[qi][:], pg_off + j * NSP * PGE, [(0, HD), (PGE, NSP), (1, K)]),
                        op=mult,
                    )

                # d[c,hd,s] = sum_k u: one packed bf16 halving, then reduce,
                # then add the k=48 leftover lane (fp32)
                a1 = dp.tile([128, CH * 36 * 24], bf16, tag="a1")
                nc.vector.tensor_tensor(
                    out=sap(a1[:], 0, [(864, CH), (24, 36), (1, 24)]),
                    in0=sap(u4[:], 0, [(U_SZ, CH), (K, 36), (1, 24)]),
                    in1=sap(u4[:], 24, [(U_SZ, CH), (K, 36), (1, 24)]),
                    op=add,
                )
                a2 = dp.tile([128, CH * 36 * 12], bf16, tag="a2")
                nc.vector.tensor_tensor(
                    out=sap(a2[:], 0, [(432, CH), (12, 36), (1, 12)]),
                    in0=sap(a1[:], 0, [(864, CH), (24, 36), (1, 12)]),
                    in1=sap(a1[:], 12, [(864, CH), (24, 36), (1, 12)]),
                    op=add,
                )
                d0 = sp.tile([128, CH * HD * NSP], f32, tag="d0")
                nc.vector.reduce_sum(
                    out=sap(d0[:], 0, [(36, CH), (1, 36)]),
                    in_=sap(a2[:], 0, [(432, CH), (12, 36), (1, 12)]),
                    axis=mybir.AxisListType.X,
                )
                d4 = sp.tile([128, CH * HD * NSP], f32, tag="d")
                nc.vector.tensor_tensor(
                    out=sap(d4[:], 0, [(36, CH), (1, 36)]),
                    in0=sap(d0[:], 0, [(36, CH), (1, 36)]),
                    in1=sap(u4[:], 48, [(U_SZ, CH), (K, 36)]),
                    op=add,
                )

                # r = 1/d fast approx; v[c,hd,s] = r * pi[c,s]  (bf16)
                r4 = sp.tile([128, CH * HD * NSP], f32, tag="r")
                nc.vector.reciprocal_approx_fast(r4[:], d4[:])
                v4 = sp.tile([128, CH * HD * NSP], bf16, tag="v")
                nc.vector.tensor_tensor(
                    out=sap(v4[:], 0, [(36, CH), (NSP, HD), (1, NSP)]),
                    in0=sap(r4[:], 0, [(36, CH), (NSP, HD), (1, NSP)]),
                    in1=sap(pg_q[qi][:], pg_off + K, [(NSP * PGE, CH), (0, HD), (PGE, NSP)]),
                    op=mult,
                )

                # vk[c,hd,s,k] = v broadcast along k, on the Act engine
                wk4 = acp.tile([128, CH * U_SZ], bf16, tag="wk")
                nc.scalar.activation(
                    sap(wk4[:], 0, [(K, CH * 36), (1, K)]),
                    sap(v4[:], 0, [(1, CH * 36), (0, K)]),
                    mybir.ActivationFunctionType.Copy,
                )
                # o-tree of the previous chunk fills the wk wait window
                if pend is not None:
                    emit_tail(pend)

                # acc[c,hd,s,k] = u * vk  (bf16 2x on DVE)
                acc4 = acp.tile([128, CH * U_SZ], bf16, tag="acc")
                nc.vector.tensor_tensor(
                    out=sap(acc4[:], 0, [(1, CH * U_SZ)]),
                    in0=sap(u4[:], 0, [(1, CH * U_SZ)]),
                    in1=sap(wk4[:], 0, [(1, CH * U_SZ)]),
                    op=mult,
                )
                pend = (acc4, wl0)
            emit_tail(pend)
    nc.compile()
    return nc


def _host_prep(attn, sims, sinds):
    from concourse import mybir

    bf_np = mybir.dt.np(mybir.dt.bfloat16)
    hj = (np.clip(np.arange(H) - KS // 2, 0, H - KS)[:, None] + np.arange(KS)[None, :])
    wj = (np.clip(np.arange(W) - KS // 2, 0, W - KS)[:, None] + np.arange(KS)[None, :])
    harange = np.arange(H)
    in_maps = []
    for b in range(B):
        sims_b = sims[b]                                  # (S,H,W)
        for q in range(4):
            cols = np.arange(WQ * q, WQ * (q + 1))
            attn2 = np.exp(np.ascontiguousarray(
                attn[b][:, :, cols, :].transpose(1, 2, 0, 3)
            )).reshape(128, WQ * HD_K).astype(bf_np)

            g = sinds[b][:, cols, :]                      # (H,WQ,9)
            patch = sims_b[
                g[:, :, :, None, None],
                hj[:, None, None, :, None],
                wj[cols][None, :, None, None, :],
            ]                                             # (H,WQ,9,7,7)
            pi = sims_b[g, harange[:, None, None], cols[None, :, None]]
            pg = np.empty((H, WQ, NSP, PGE), dtype=np.float32)
            pg[..., :K] = patch.reshape(H, WQ, NSP, K)
            pg[..., K] = pi
            in_maps.append({
                "attn": attn2,
                "pg": pg.reshape(128, WQ * NSP * PGE).astype(bf_np),
            })
    return in_maps


def kernel(attn, sims, sinds):
    from concourse.bass_utils import run_bass_kernel_spmd

    attn = np.asarray(attn, dtype=np.float32)
    sims = np.asarray(sims, dtype=np.float32)
    sinds = np.asarray(sinds)

    if "nc" not in _cached:
        _cached["nc"] = _build()
    nc = _cached["nc"]

    in_maps = _host_prep(attn, sims, sinds)
    res = run_bass_kernel_spmd(nc, in_maps, list(range(N_CORES)))

    out = np.empty((B, HD, H, W, K), dtype=np.float32)
    for cid in range(N_CORES):
        b, q = divmod(cid, 4)
        o = res.results[cid]["out"].astype(np.float32).reshape(H, WQ, HD, K)
        out[b][:, :, WQ * q:WQ * (q + 1), :] = o.transpose(2, 0, 1, 3)
    return out


# revision 24
# speedup vs baseline: 1.0919x; 1.0631x over previous
import sys

if "/opt/trn_rl_repo" not in sys.path:
    sys.path.insert(0, "/opt/trn_rl_repo")

import numpy as np

B, HD, H, W, K = 2, 4, 128, 128, 49
KS = 7
NSP = 9
S = 64
N_CORES = 8
WQ = W // 4            # 32 columns per core
PGE = 50               # per-(pixel,s) gathered element: 49 patch + pi
NQ = 8                 # input DMA groups (one per chunk)
CPQ = WQ // NQ         # columns per quarter
CH = 4                 # columns per compute chunk
NCH = WQ // CH         # chunks
HD_K = HD * K          # 196
U_SZ = HD * NSP * K    # 1764

_cached = {}


def _build():
    import concourse.bass as bass
    import concourse.tile as tile
    from concourse import bacc, mybir

    f32 = mybir.dt.float32
    bf16 = mybir.dt.bfloat16
    mult = mybir.AluOpType.mult
    add = mybir.AluOpType.add

    nc = bacc.Bacc("TRN2", target_bir_lowering=False, debug=False, num_devices=N_CORES)
    # e2[h, wl, hd, k] = exp(logits) (bf16, host precomputed)
    attn_s = nc.dram_tensor("attn", [128, WQ * HD_K], bf16, kind="ExternalInput")
    # pg[h, wl, s, 0:49]=patch, [...,49]=pi (bf16, host pre-gathered)
    pg_s = nc.dram_tensor("pg", [128, WQ * NSP * PGE], bf16, kind="ExternalInput")
    # out[h, wl, hd, k] f32
    out_s = nc.dram_tensor("out", [128, WQ * HD_K], bf16, kind="ExternalOutput")

    def ap(t, off, dims):
        return bass.AP(t, off, [list(d) for d in dims])

    def sap(tap, extra_off, dims):
        return bass.AP(tap.tensor, tap.offset + extra_off, [list(tap.ap[0]), *[list(d) for d in dims]])

    with tile.TileContext(nc) as tc:
        with (
            tc.tile_pool(name="inq", bufs=NQ) as inq,
            tc.tile_pool(name="ep", bufs=2) as epool,
            tc.tile_pool(name="up", bufs=3) as up,
            tc.tile_pool(name="sp", bufs=4) as sp,
            tc.tile_pool(name="dp", bufs=3) as dp,
            tc.tile_pool(name="acp", bufs=2) as acp,
            tc.tile_pool(name="tp", bufs=2) as tp,
            tc.tile_pool(name="op", bufs=3) as op,
        ):
            at_q = []
            pg_q = []
            for qi in range(NQ):
                at_t = inq.tile([128, CPQ * HD_K], bf16, tag="atq")
                nc.sync.dma_start(
                    at_t[:],
                    ap(attn_s, qi * CPQ * HD_K, [(WQ * HD_K, 128), (1, CPQ * HD_K)]),
                )
                pg_t = inq.tile([128, CPQ * NSP * PGE], bf16, tag="pgq")
                nc.sync.dma_start(
                    pg_t[:],
                    ap(pg_s, qi * CPQ * NSP * PGE, [(WQ * NSP * PGE, 128), (1, CPQ * NSP * PGE)]),
                )
                at_q.append(at_t)
                pg_q.append(pg_t)

            def emit_tail(st):
                acc4, wl0 = st
                # o[hd,k] = sum_s acc via packed bf16 add-tree
                t1 = tp.tile([128, CH * HD * 4 * K], bf16, tag="t1")
                nc.vector.tensor_tensor(
                    out=sap(t1[:], 0, [(784, CH), (196, HD), (1, 4 * K)]),
                    in0=sap(acc4[:], 0, [(U_SZ, CH), (441, HD), (1, 4 * K)]),
                    in1=sap(acc4[:], 4 * K, [(U_SZ, CH), (441, HD), (1, 4 * K)]),
                    op=add,
                )
                t2 = tp.tile([128, CH * HD * 2 * K], bf16, tag="t2")
                nc.vector.tensor_tensor(
                    out=sap(t2[:], 0, [(392, CH), (98, HD), (1, 2 * K)]),
                    in0=sap(t1[:], 0, [(784, CH), (196, HD), (1, 2 * K)]),
                    in1=sap(t1[:], 2 * K, [(784, CH), (196, HD), (1, 2 * K)]),
                    op=add,
                )
                t3 = tp.tile([128, CH * HD_K], bf16, tag="t3")
                nc.vector.tensor_tensor(
                    out=sap(t3[:], 0, [(196, CH), (49, HD), (1, K)]),
                    in0=sap(t2[:], 0, [(392, CH), (98, HD), (1, K)]),
                    in1=sap(t2[:], K, [(392, CH), (98, HD), (1, K)]),
                    op=add,
                )
                o4 = op.tile([128, CH * HD_K], bf16, tag="o")
                nc.vector.tensor_tensor(
                    out=sap(o4[:], 0, [(196, CH), (49, HD), (1, K)]),
                    in0=sap(t3[:], 0, [(196, CH), (49, HD), (1, K)]),
                    in1=sap(acc4[:], 8 * K, [(U_SZ, CH), (441, HD), (1, K)]),
                    op=add,
                )
                nc.sync.dma_start(
                    ap(out_s, wl0 * HD_K, [(WQ * HD_K, 128), (HD_K, CH), (1, HD_K)]),
                    o4[:],
                )

            def head(ch):
                wl0 = ch * CH
                qi, jo = divmod(wl0, CPQ)
                at_off = jo * HD_K
                pg_off = jo * NSP * PGE

                # u[c,hd,s,k] = e[c,hd,k] * p[c,s,k]  (bf16 2x; per-column —
                # the broadcast dims don't fit the 3-free-dim ISA limit)
                u4 = up.tile([128, CH * U_SZ], bf16, tag="u")
                for j in range(CH):
                    nc.vector.tensor_tensor(
                        out=sap(u4[:], j * U_SZ, [(441, HD), (K, NSP), (1, K)]),
                        in0=sap(at_q[qi][:], at_off + j * HD_K, [(K, HD), (0, NSP), (1, K)]),
                        in1=sap(pg_q[qi][:], pg_off + j * NSP * PGE, [(0, HD), (PGE, NSP), (1, K)]),
                        op=mult,
                    )

                # d[c,hd,s] = sum_k u: one packed bf16 halving, then reduce,
                # then add the k=48 leftover lane (fp32)
                a1 = dp.tile([128, CH * 36 * 24], bf16, tag="a1")
                nc.vector.tensor_tensor(
                    out=sap(a1[:], 0, [(864, CH), (24, 36), (1, 24)]),
                    in0=sap(u4[:], 0, [(U_SZ, CH), (K, 36), (1, 24)]),
                    in1=sap(u4[:], 24, [(U_SZ, CH), (K, 36), (1, 24)]),
                    op=add,
                )
                a2 = dp.tile([128, CH * 36 * 12], bf16, tag="a2")
                nc.vector.tensor_tensor(
                    out=sap(a2[:], 0, [(432, CH), (12, 36), (1, 12)]),
                    in0=sap(a1[:], 0, [(864, CH), (24, 36), (1, 12)]),
                    in1=sap(a1[:], 12, [(864, CH), (24, 36), (1, 12)]),
                    op=add,
                )
                d0 = sp.tile([128, CH * HD * NSP], f32, tag="d0")
                nc.vector.reduce_sum(
                    out=sap(d0[:], 0, [(36, CH), (1, 36)]),
                    in_=sap(a2[:], 0, [(432, CH), (12, 36), (1, 12)]),
                    axis=mybir.AxisListType.X,
                )
                d4 = sp.tile([128, CH * HD * NSP], f32, tag="d")
                nc.vector.tensor_tensor(
                    out=sap(d4[:], 0, [(36, CH), (1, 36)]),
                    in0=sap(d0[:], 0, [(36, CH), (1, 36)]),
                    in1=sap(u4[:], 48, [(U_SZ, CH), (K, 36)]),
                    op=add,
                )

                # r = 1/d fast approx; v[c,hd,s] = r * pi[c,s]  (bf16)
                r4 = sp.tile([128, CH * HD * NSP], f32, tag="r")
                nc.vector.reciprocal_approx_fast(r4[:], d4[:])
                v4 = sp.tile([128, CH * HD * NSP], bf16, tag="v")
                nc.vector.tensor_tensor(
                    out=sap(v4[:], 0, [(36, CH), (NSP, HD), (1, NSP)]),
                    in0=sap(r4[:], 0, [(36, CH), (NSP, HD), (1, NSP)]),
                    in1=sap(pg_q[qi][:], pg_off + K, [(NSP * PGE, CH), (0, HD), (PGE, NSP)]),
                    op=mult,
                )

                # vk[c,hd,s,k] = v broadcast along k, on the Act engine
                wk4 = acp.tile([128, CH * U_SZ], bf16, tag="wk")
                nc.scalar.activation(
                    sap(wk4[:], 0, [(K, CH * 36), (1, K)]),
                    sap(v4[:], 0, [(1, CH * 36), (0, K)]),
                    mybir.ActivationFunctionType.Copy,
                )
                return u4, wk4, wl0

            def emit_acc(h):
                u4, wk4, wl0 = h
                # acc[c,hd,s,k] = u * vk  (bf16 2x on DVE)
                acc4 = acp.tile([128, CH * U_SZ], bf16, tag="acc")
                nc.vector.tensor_tensor(
                    out=sap(acc4[:], 0, [(1, CH * U_SZ)]),
                    in0=sap(u4[:], 0, [(1, CH * U_SZ)]),
                    in1=sap(wk4[:], 0, [(1, CH * U_SZ)]),
                    op=mult,
                )
                return (acc4, wl0)

            # 2-deep software pipeline: wk(i) completes a full chunk ahead
            # of acc(i); tree(i-1) trails.
            heads = [head(0), head(1)]
            pend = None
            for ch in range(NCH):
                st = emit_acc(heads[ch % 2])
                if ch + 2 < NCH:
                    heads[ch % 2] = head(ch + 2)
                if pend is not None:
                    emit_tail(pend)
                pend = st
            emit_tail(pend)
    nc.compile()
    return nc


def _host_prep(attn, sims, sinds):
    from concourse import mybir

    bf_np = mybir.dt.np(mybir.dt.bfloat16)
    hj = (np.clip(np.arange(H) - KS // 2, 0, H - KS)[:, None] + np.arange(KS)[None, :])
    wj = (np.clip(np.arange(W) - KS // 2, 0, W - KS)[:, None] + np.arange(KS)[None, :])
    harange = np.arange(H)
    in_maps = []
    for b in range(B):
        sims_b = sims[b]                                  # (S,H,W)
        for q in range(4):
            cols = np.arange(WQ * q, WQ * (q + 1))
            attn2 = np.exp(np.ascontiguousarray(
                attn[b][:, :, cols, :].transpose(1, 2, 0, 3)
            )).reshape(128, WQ * HD_K).astype(bf_np)

            g = sinds[b][:, cols, :]                      # (H,WQ,9)
            patch = sims_b[
                g[:, :, :, None, None],
                hj[:, None, None, :, None],
                wj[cols][None, :, None, None, :],
            ]                                             # (H,WQ,9,7,7)
            pi = sims_b[g, harange[:, None, None], cols[None, :, None]]
            pg = np.empty((H, WQ, NSP, PGE), dtype=np.float32)
            pg[..., :K] = patch.reshape(H, WQ, NSP, K)
            pg[..., K] = pi
            in_maps.append({
                "attn": attn2,
                "pg": pg.reshape(128, WQ * NSP * PGE).astype(bf_np),
            })
    return in_maps


def kernel(attn, sims, sinds):
    from concourse.bass_utils import run_bass_kernel_spmd

    attn = np.asarray(attn, dtype=np.float32)
    sims = np.asarray(sims, dtype=np.float32)
    sinds = np.asarray(sinds)

    if "nc" not in _cached:
        _cached["nc"] = _build()
    nc = _cached["nc"]

    in_maps = _host_prep(attn, sims, sinds)
    res = run_bass_kernel_spmd(nc, in_maps, list(range(N_CORES)))

    out = np.empty((B, HD, H, W, K), dtype=np.float32)
    for cid in range(N_CORES):
        b, q = divmod(cid, 4)
        o = res.results[cid]["out"].astype(np.float32).reshape(H, WQ, HD, K)
        out[b][:, :, WQ * q:WQ * (q + 1), :] = o.transpose(2, 0, 1, 3)
    return out


# revision 25
# speedup vs baseline: 1.0935x; 1.0015x over previous
import sys

if "/opt/trn_rl_repo" not in sys.path:
    sys.path.insert(0, "/opt/trn_rl_repo")

import numpy as np

B, HD, H, W, K = 2, 4, 128, 128, 49
KS = 7
NSP = 9
S = 64
N_CORES = 8
WQ = W // 4            # 32 columns per core
PGE = 50               # per-(pixel,s) gathered element: 49 patch + pi
NQ = 8                 # input DMA groups (one per chunk)
CPQ = WQ // NQ         # columns per quarter
CH = 4                 # columns per compute chunk
NCH = WQ // CH         # chunks
HD_K = HD * K          # 196
U_SZ = HD * NSP * K    # 1764

_cached = {}


def _build():
    import concourse.bass as bass
    import concourse.tile as tile
    from concourse import bacc, mybir

    f32 = mybir.dt.float32
    bf16 = mybir.dt.bfloat16
    mult = mybir.AluOpType.mult
    add = mybir.AluOpType.add

    nc = bacc.Bacc("TRN2", target_bir_lowering=False, debug=False, num_devices=N_CORES)
    # e2[h, wl, hd, k] = exp(logits) (bf16, host precomputed)
    attn_s = nc.dram_tensor("attn", [128, WQ * HD_K], bf16, kind="ExternalInput")
    # pg[h, wl, s, 0:49]=patch, [...,49]=pi (bf16, host pre-gathered)
    pg_s = nc.dram_tensor("pg", [128, WQ * NSP * PGE], bf16, kind="ExternalInput")
    # out[h, wl, hd, k] f32
    out_s = nc.dram_tensor("out", [128, WQ * HD_K], bf16, kind="ExternalOutput")

    def ap(t, off, dims):
        return bass.AP(t, off, [list(d) for d in dims])

    def sap(tap, extra_off, dims):
        return bass.AP(tap.tensor, tap.offset + extra_off, [list(tap.ap[0]), *[list(d) for d in dims]])

    with tile.TileContext(nc) as tc:
        with (
            tc.tile_pool(name="inq", bufs=NQ) as inq,
            tc.tile_pool(name="ep", bufs=2) as epool,
            tc.tile_pool(name="up", bufs=3) as up,
            tc.tile_pool(name="sp", bufs=4) as sp,
            tc.tile_pool(name="dp", bufs=3) as dp,
            tc.tile_pool(name="acp", bufs=2) as acp,
            tc.tile_pool(name="tp", bufs=2) as tp,
            tc.tile_pool(name="op", bufs=3) as op,
        ):
            at_q = [None] * NQ
            pg_q = [None] * NQ

            def load_in(qi):
                # pg first (bigger transfer), then attn
                pg_t = inq.tile([128, CPQ * NSP * PGE], bf16, tag="pgq")
                nc.sync.dma_start(
                    pg_t[:],
                    ap(pg_s, qi * CPQ * NSP * PGE, [(WQ * NSP * PGE, 128), (1, CPQ * NSP * PGE)]),
                )
                at_t = inq.tile([128, CPQ * HD_K], bf16, tag="atq")
                nc.sync.dma_start(
                    at_t[:],
                    ap(attn_s, qi * CPQ * HD_K, [(WQ * HD_K, 128), (1, CPQ * HD_K)]),
                )
                at_q[qi] = at_t
                pg_q[qi] = pg_t

            for qi in range(3):
                load_in(qi)

            def emit_tail(st):
                acc4, wl0 = st
                # o[hd,k] = sum_s acc via packed bf16 add-tree
                t1 = tp.tile([128, CH * HD * 4 * K], bf16, tag="t1")
                nc.vector.tensor_tensor(
                    out=sap(t1[:], 0, [(784, CH), (196, HD), (1, 4 * K)]),
                    in0=sap(acc4[:], 0, [(U_SZ, CH), (441, HD), (1, 4 * K)]),
                    in1=sap(acc4[:], 4 * K, [(U_SZ, CH), (441, HD), (1, 4 * K)]),
                    op=add,
                )
                t2 = tp.tile([128, CH * HD * 2 * K], bf16, tag="t2")
                nc.vector.tensor_tensor(
                    out=sap(t2[:], 0, [(392, CH), (98, HD), (1, 2 * K)]),
                    in0=sap(t1[:], 0, [(784, CH), (196, HD), (1, 2 * K)]),
                    in1=sap(t1[:], 2 * K, [(784, CH), (196, HD), (1, 2 * K)]),
                    op=add,
                )
                t3 = tp.tile([128, CH * HD_K], bf16, tag="t3")
                nc.vector.tensor_tensor(
                    out=sap(t3[:], 0, [(196, CH), (49, HD), (1, K)]),
                    in0=sap(t2[:], 0, [(392, CH), (98, HD), (1, K)]),
                    in1=sap(t2[:], K, [(392, CH), (98, HD), (1, K)]),
                    op=add,
                )
                o4 = op.tile([128, CH * HD_K], bf16, tag="o")
                nc.vector.tensor_tensor(
                    out=sap(o4[:], 0, [(196, CH), (49, HD), (1, K)]),
                    in0=sap(t3[:], 0, [(196, CH), (49, HD), (1, K)]),
                    in1=sap(acc4[:], 8 * K, [(U_SZ, CH), (441, HD), (1, K)]),
                    op=add,
                )
                nc.sync.dma_start(
                    ap(out_s, wl0 * HD_K, [(WQ * HD_K, 128), (HD_K, CH), (1, HD_K)]),
                    o4[:],
                )

            def head(ch):
                wl0 = ch * CH
                qi, jo = divmod(wl0, CPQ)
                at_off = jo * HD_K
                pg_off = jo * NSP * PGE

                # u[c,hd,s,k] = e[c,hd,k] * p[c,s,k]  (bf16 2x; per-column —
                # the broadcast dims don't fit the 3-free-dim ISA limit)
                u4 = up.tile([128, CH * U_SZ], bf16, tag="u")
                for j in range(CH):
                    nc.vector.tensor_tensor(
                        out=sap(u4[:], j * U_SZ, [(441, HD), (K, NSP), (1, K)]),
                        in0=sap(at_q[qi][:], at_off + j * HD_K, [(K, HD), (0, NSP), (1, K)]),
                        in1=sap(pg_q[qi][:], pg_off + j * NSP * PGE, [(0, HD), (PGE, NSP), (1, K)]),
                        op=mult,
                    )

                # d[c,hd,s] = sum_k u: one packed bf16 halving, then reduce,
                # then add the k=48 leftover lane (fp32)
                a1 = dp.tile([128, CH * 36 * 24], bf16, tag="a1")
                nc.vector.tensor_tensor(
                    out=sap(a1[:], 0, [(864, CH), (24, 36), (1, 24)]),
                    in0=sap(u4[:], 0, [(U_SZ, CH), (K, 36), (1, 24)]),
                    in1=sap(u4[:], 24, [(U_SZ, CH), (K, 36), (1, 24)]),
                    op=add,
                )
                a2 = dp.tile([128, CH * 36 * 12], bf16, tag="a2")
                nc.vector.tensor_tensor(
                    out=sap(a2[:], 0, [(432, CH), (12, 36), (1, 12)]),
                    in0=sap(a1[:], 0, [(864, CH), (24, 36), (1, 12)]),
                    in1=sap(a1[:], 12, [(864, CH), (24, 36), (1, 12)]),
                    op=add,
                )
                d0 = sp.tile([128, CH * HD * NSP], f32, tag="d0")
                nc.vector.reduce_sum(
                    out=sap(d0[:], 0, [(36, CH), (1, 36)]),
                    in_=sap(a2[:], 0, [(432, CH), (12, 36), (1, 12)]),
                    axis=mybir.AxisListType.X,
                )
                d4 = sp.tile([128, CH * HD * NSP], f32, tag="d")
                nc.vector.tensor_tensor(
                    out=sap(d4[:], 0, [(36, CH), (1, 36)]),
                    in0=sap(d0[:], 0, [(36, CH), (1, 36)]),
                    in1=sap(u4[:], 48, [(U_SZ, CH), (K, 36)]),
                    op=add,
                )

                # r = 1/d fast approx; v[c,hd,s] = r * pi[c,s]  (bf16)
                r4 = sp.tile([128, CH * HD * NSP], f32, tag="r")
                nc.vector.reciprocal_approx_fast(r4[:], d4[:])
                v4 = sp.tile([128, CH * HD * NSP], bf16, tag="v")
                nc.vector.tensor_tensor(
                    out=sap(v4[:], 0, [(36, CH), (NSP, HD), (1, NSP)]),
                    in0=sap(r4[:], 0, [(36, CH), (NSP, HD), (1, NSP)]),
                    in1=sap(pg_q[qi][:], pg_off + K, [(NSP * PGE, CH), (0, HD), (PGE, NSP)]),
                    op=mult,
                )

                # vk[c,hd,s,k] = v broadcast along k, on the Act engine
                wk4 = acp.tile([128, CH * U_SZ], bf16, tag="wk")
                nc.scalar.activation(
                    sap(wk4[:], 0, [(K, CH * 36), (1, K)]),
                    sap(v4[:], 0, [(1, CH * 36), (0, K)]),
                    mybir.ActivationFunctionType.Copy,
                )
                return u4, wk4, wl0

            def emit_acc(h):
                u4, wk4, wl0 = h
                # acc[c,hd,s,k] = u * vk  (bf16 2x on DVE)
                acc4 = acp.tile([128, CH * U_SZ], bf16, tag="acc")
                nc.vector.tensor_tensor(
                    out=sap(acc4[:], 0, [(1, CH * U_SZ)]),
                    in0=sap(u4[:], 0, [(1, CH * U_SZ)]),
                    in1=sap(wk4[:], 0, [(1, CH * U_SZ)]),
                    op=mult,
                )
                return (acc4, wl0)

            # 2-deep software pipeline: wk(i) completes a full chunk ahead
            # of acc(i); tree(i-1) trails.
            heads = [head(0), head(1)]
            pend = None
            for ch in range(NCH):
                if ch + 3 < NCH:
                    load_in(ch + 3)
                st = emit_acc(heads[ch % 2])
                if ch + 2 < NCH:
                    heads[ch % 2] = head(ch + 2)
                if pend is not None:
                    emit_tail(pend)
                pend = st
            emit_tail(pend)
    nc.compile()
    return nc


def _host_prep(attn, sims, sinds):
    from concourse import mybir

    bf_np = mybir.dt.np(mybir.dt.bfloat16)
    hj = (np.clip(np.arange(H) - KS // 2, 0, H - KS)[:, None] + np.arange(KS)[None, :])
    wj = (np.clip(np.arange(W) - KS // 2, 0, W - KS)[:, None] + np.arange(KS)[None, :])
    harange = np.arange(H)
    in_maps = []
    for b in range(B):
        sims_b = sims[b]                                  # (S,H,W)
        for q in range(4):
            cols = np.arange(WQ * q, WQ * (q + 1))
            attn2 = np.exp(np.ascontiguousarray(
                attn[b][:, :, cols, :].transpose(1, 2, 0, 3)
            )).reshape(128, WQ * HD_K).astype(bf_np)

            g = sinds[b][:, cols, :]                      # (H,WQ,9)
            patch = sims_b[
                g[:, :, :, None, None],
                hj[:, None, None, :, None],
                wj[cols][None, :, None, None, :],
            ]                                             # (H,WQ,9,7,7)
            pi = sims_b[g, harange[:, None, None], cols[None, :, None]]
            pg = np.empty((H, WQ, NSP, PGE), dtype=np.float32)
            pg[..., :K] = patch.reshape(H, WQ, NSP, K)
            pg[..., K] = pi
            in_maps.append({
                "attn": attn2,
                "pg": pg.reshape(128, WQ * NSP * PGE).astype(bf_np),
            })
    return in_maps


def kernel(attn, sims, sinds):
    from concourse.bass_utils import run_bass_kernel_spmd

    attn = np.asarray(attn, dtype=np.float32)
    sims = np.asarray(sims, dtype=np.float32)
    sinds = np.asarray(sinds)

    if "nc" not in _cached:
        _cached["nc"] = _build()
    nc = _cached["nc"]

    in_maps = _host_prep(attn, sims, sinds)
    res = run_bass_kernel_spmd(nc, in_maps, list(range(N_CORES)))

    out = np.empty((B, HD, H, W, K), dtype=np.float32)
    for cid in range(N_CORES):
        b, q = divmod(cid, 4)
        o = res.results[cid]["out"].astype(np.float32).reshape(H, WQ, HD, K)
        out[b][:, :, WQ * q:WQ * (q + 1), :] = o.transpose(2, 0, 1, 3)
    return out


# revision 26
# speedup vs baseline: 1.1082x; 1.0135x over previous
import sys

if "/opt/trn_rl_repo" not in sys.path:
    sys.path.insert(0, "/opt/trn_rl_repo")

import numpy as np

B, HD, H, W, K = 2, 4, 128, 128, 49
KS = 7
NSP = 9
S = 64
N_CORES = 8
WQ = W // 4            # 32 columns per core
PGE = 50               # per-(pixel,s) gathered element: 49 patch + pi
NQ = 8                 # input DMA groups (one per chunk)
CPQ = WQ // NQ         # columns per quarter
CH = 4                 # columns per compute chunk
NCH = WQ // CH         # chunks
HD_K = HD * K          # 196
U_SZ = HD * NSP * K    # 1764

_cached = {}


def _build():
    import concourse.bass as bass
    import concourse.tile as tile
    from concourse import bacc, mybir

    f32 = mybir.dt.float32
    bf16 = mybir.dt.bfloat16
    mult = mybir.AluOpType.mult
    add = mybir.AluOpType.add

    nc = bacc.Bacc("TRN2", target_bir_lowering=False, debug=False, num_devices=N_CORES)
    # e2[h, wl, hd, k] = exp(logits) (bf16, host precomputed)
    attn_s = nc.dram_tensor("attn", [128, WQ * HD_K], bf16, kind="ExternalInput")
    # pg[h, wl, s, 0:49]=patch, [...,49]=pi (bf16, host pre-gathered)
    pg_s = nc.dram_tensor("pg", [128, WQ * NSP * PGE], bf16, kind="ExternalInput")
    # out[h, wl, hd, k] f32
    out_s = nc.dram_tensor("out", [128, WQ * HD_K], bf16, kind="ExternalOutput")

    def ap(t, off, dims):
        return bass.AP(t, off, [list(d) for d in dims])

    def sap(tap, extra_off, dims):
        return bass.AP(tap.tensor, tap.offset + extra_off, [list(tap.ap[0]), *[list(d) for d in dims]])

    with tile.TileContext(nc) as tc:
        with (
            tc.tile_pool(name="inq", bufs=NQ) as inq,
            tc.tile_pool(name="ep", bufs=2) as epool,
            tc.tile_pool(name="up", bufs=3) as up,
            tc.tile_pool(name="sp", bufs=4) as sp,
            tc.tile_pool(name="dp", bufs=3) as dp,
            tc.tile_pool(name="acp", bufs=2) as acp,
            tc.tile_pool(name="tp", bufs=2) as tp,
            tc.tile_pool(name="op", bufs=3) as op,
        ):
            at_q = [None] * NQ
            pg_q = [None] * NQ

            def load_in(qi):
                # pg first (bigger transfer), then attn
                pg_t = inq.tile([128, CPQ * NSP * PGE], bf16, tag="pgq")
                nc.sync.dma_start(
                    pg_t[:],
                    ap(pg_s, qi * CPQ * NSP * PGE, [(WQ * NSP * PGE, 128), (1, CPQ * NSP * PGE)]),
                )
                at_t = inq.tile([128, CPQ * HD_K], bf16, tag="atq")
                nc.sync.dma_start(
                    at_t[:],
                    ap(attn_s, qi * CPQ * HD_K, [(WQ * HD_K, 128), (1, CPQ * HD_K)]),
                )
                at_q[qi] = at_t
                pg_q[qi] = pg_t

            for qi in range(3):
                load_in(qi)

            def emit_tail(st):
                acc4, wl0 = st
                # o[hd,k] = sum_s acc via packed bf16 add-tree
                t1 = tp.tile([128, CH * HD * 4 * K], bf16, tag="t1")
                nc.vector.tensor_tensor(
                    out=sap(t1[:], 0, [(784, CH), (196, HD), (1, 4 * K)]),
                    in0=sap(acc4[:], 0, [(U_SZ, CH), (441, HD), (1, 4 * K)]),
                    in1=sap(acc4[:], 4 * K, [(U_SZ, CH), (441, HD), (1, 4 * K)]),
                    op=add,
                )
                t2 = tp.tile([128, CH * HD * 2 * K], bf16, tag="t2")
                nc.vector.tensor_tensor(
                    out=sap(t2[:], 0, [(392, CH), (98, HD), (1, 2 * K)]),
                    in0=sap(t1[:], 0, [(784, CH), (196, HD), (1, 2 * K)]),
                    in1=sap(t1[:], 2 * K, [(784, CH), (196, HD), (1, 2 * K)]),
                    op=add,
                )
                t3 = tp.tile([128, CH * HD_K], bf16, tag="t3")
                nc.vector.tensor_tensor(
                    out=sap(t3[:], 0, [(196, CH), (49, HD), (1, K)]),
                    in0=sap(t2[:], 0, [(392, CH), (98, HD), (1, K)]),
                    in1=sap(t2[:], K, [(392, CH), (98, HD), (1, K)]),
                    op=add,
                )
                o4 = op.tile([128, CH * HD_K], bf16, tag="o")
                nc.vector.tensor_tensor(
                    out=sap(o4[:], 0, [(196, CH), (49, HD), (1, K)]),
                    in0=sap(t3[:], 0, [(196, CH), (49, HD), (1, K)]),
                    in1=sap(acc4[:], 8 * K, [(U_SZ, CH), (441, HD), (1, K)]),
                    op=add,
                )
                nc.sync.dma_start(
                    ap(out_s, wl0 * HD_K, [(WQ * HD_K, 128), (HD_K, CH), (1, HD_K)]),
                    o4[:],
                )

            def head(ch):
                wl0 = ch * CH
                qi, jo = divmod(wl0, CPQ)
                at_off = jo * HD_K
                pg_off = jo * NSP * PGE

                # u[c,hd,s,k] = e[c,hd,k] * p[c,s,k]  (bf16 2x; per-column —
                # the broadcast dims don't fit the 3-free-dim ISA limit)
                u4 = up.tile([128, CH * U_SZ], bf16, tag="u")
                for j in range(CH):
                    nc.vector.tensor_tensor(
                        out=sap(u4[:], j * U_SZ, [(441, HD), (K, NSP), (1, K)]),
                        in0=sap(at_q[qi][:], at_off + j * HD_K, [(K, HD), (0, NSP), (1, K)]),
                        in1=sap(pg_q[qi][:], pg_off + j * NSP * PGE, [(0, HD), (PGE, NSP), (1, K)]),
                        op=mult,
                    )

                # d[c,hd,s] = sum_k u: one packed bf16 halving, then reduce,
                # then add the k=48 leftover lane (fp32)
                a1 = dp.tile([128, CH * 36 * 24], bf16, tag="a1")
                nc.vector.tensor_tensor(
                    out=sap(a1[:], 0, [(864, CH), (24, 36), (1, 24)]),
                    in0=sap(u4[:], 0, [(U_SZ, CH), (K, 36), (1, 24)]),
                    in1=sap(u4[:], 24, [(U_SZ, CH), (K, 36), (1, 24)]),
                    op=add,
                )
                a2 = dp.tile([128, CH * 36 * 12], bf16, tag="a2")
                nc.vector.tensor_tensor(
                    out=sap(a2[:], 0, [(432, CH), (12, 36), (1, 12)]),
                    in0=sap(a1[:], 0, [(864, CH), (24, 36), (1, 12)]),
                    in1=sap(a1[:], 12, [(864, CH), (24, 36), (1, 12)]),
                    op=add,
                )
                a3 = dp.tile([128, CH * 36 * 6], bf16, tag="a3")
                nc.vector.tensor_tensor(
                    out=sap(a3[:], 0, [(216, CH), (6, 36), (1, 6)]),
                    in0=sap(a2[:], 0, [(432, CH), (12, 36), (1, 6)]),
                    in1=sap(a2[:], 6, [(432, CH), (12, 36), (1, 6)]),
                    op=add,
                )
                d0 = sp.tile([128, CH * HD * NSP], f32, tag="d0")
                nc.vector.reduce_sum(
                    out=sap(d0[:], 0, [(36, CH), (1, 36)]),
                    in_=sap(a3[:], 0, [(216, CH), (6, 36), (1, 6)]),
                    axis=mybir.AxisListType.X,
                )
                d4 = sp.tile([128, CH * HD * NSP], f32, tag="d")
                nc.vector.tensor_tensor(
                    out=sap(d4[:], 0, [(36, CH), (1, 36)]),
                    in0=sap(d0[:], 0, [(36, CH), (1, 36)]),
                    in1=sap(u4[:], 48, [(U_SZ, CH), (K, 36)]),
                    op=add,
                )

                # r = 1/d fast approx; v[c,hd,s] = r * pi[c,s]  (bf16)
                r4 = sp.tile([128, CH * HD * NSP], f32, tag="r")
                nc.vector.reciprocal_approx_fast(r4[:], d4[:])
                v4 = sp.tile([128, CH * HD * NSP], bf16, tag="v")
                nc.vector.tensor_tensor(
                    out=sap(v4[:], 0, [(36, CH), (NSP, HD), (1, NSP)]),
                    in0=sap(r4[:], 0, [(36, CH), (NSP, HD), (1, NSP)]),
                    in1=sap(pg_q[qi][:], pg_off + K, [(NSP * PGE, CH), (0, HD), (PGE, NSP)]),
                    op=mult,
                )

                # vk[c,hd,s,k] = v broadcast along k, on the Act engine
                wk4 = acp.tile([128, CH * U_SZ], bf16, tag="wk")
                nc.scalar.activation(
                    sap(wk4[:], 0, [(K, CH * 36), (1, K)]),
                    sap(v4[:], 0, [(1, CH * 36), (0, K)]),
                    mybir.ActivationFunctionType.Copy,
                )
                return u4, wk4, wl0

            def emit_acc(h):
                u4, wk4, wl0 = h
                # acc[c,hd,s,k] = u * vk  (bf16 2x on DVE)
                acc4 = acp.tile([128, CH * U_SZ], bf16, tag="acc")
                nc.vector.tensor_tensor(
                    out=sap(acc4[:], 0, [(1, CH * U_SZ)]),
                    in0=sap(u4[:], 0, [(1, CH * U_SZ)]),
                    in1=sap(wk4[:], 0, [(1, CH * U_SZ)]),
                    op=mult,
                )
                return (acc4, wl0)

            # 2-deep software pipeline: wk(i) completes a full chunk ahead
            # of acc(i); tree(i-1) trails.
            heads = [head(0), head(1)]
            pend = None
            for ch in range(NCH):
                if ch + 3 < NCH:
                    load_in(ch + 3)
                st = emit_acc(heads[ch % 2])
                if ch + 2 < NCH:
                    heads[ch % 2] = head(ch + 2)
                if pend is not None:
                    emit_tail(pend)
                pend = st
            emit_tail(pend)
    nc.compile()
    return nc


def _host_prep(attn, sims, sinds):
    from concourse import mybir

    bf_np = mybir.dt.np(mybir.dt.bfloat16)
    hj = (np.clip(np.arange(H) - KS // 2, 0, H - KS)[:, None] + np.arange(KS)[None, :])
    wj = (np.clip(np.arange(W) - KS // 2, 0, W - KS)[:, None] + np.arange(KS)[None, :])
    harange = np.arange(H)
    in_maps = []
    for b in range(B):
        sims_b = sims[b]                                  # (S,H,W)
        for q in range(4):
            cols = np.arange(WQ * q, WQ * (q + 1))
            attn2 = np.exp(np.ascontiguousarray(
                attn[b][:, :, cols, :].transpose(1, 2, 0, 3)
            )).reshape(128, WQ * HD_K).astype(bf_np)

            g = sinds[b][:, cols, :]                      # (H,WQ,9)
            patch = sims_b[
                g[:, :, :, None, None],
                hj[:, None, None, :, None],
                wj[cols][None, :, None, None, :],
            ]                                             # (H,WQ,9,7,7)
            pi = sims_b[g, harange[:, None, None], cols[None, :, None]]
            pg = np.empty((H, WQ, NSP, PGE), dtype=np.float32)
            pg[..., :K] = patch.reshape(H, WQ, NSP, K)
            pg[..., K] = pi
            in_maps.append({
                "attn": attn2,
                "pg": pg.reshape(128, WQ * NSP * PGE).astype(bf_np),
            })
    return in_maps


def kernel(attn, sims, sinds):
    from concourse.bass_utils import run_bass_kernel_spmd

    attn = np.asarray(attn, dtype=np.float32)
    sims = np.asarray(sims, dtype=np.float32)
    sinds = np.asarray(sinds)

    if "nc" not in _cached:
        _cached["nc"] = _build()
    nc = _cached["nc"]

    in_maps = _host_prep(attn, sims, sinds)
    res = run_bass_kernel_spmd(nc, in_maps, list(range(N_CORES)))

    out = np.empty((B, HD, H, W, K), dtype=np.float32)
    for cid in range(N_CORES):
        b, q = divmod(cid, 4)
        o = res.results[cid]["out"].astype(np.float32).reshape(H, WQ, HD, K)
        out[b][:, :, WQ * q:WQ * (q + 1), :] = o.transpose(2, 0, 1, 3)
    return out
